# revision 15
# baseline (speedup 1.0000x reference)
"""Causal attention (B=4, L=4096, D=2048, HD=128) on 8 TRN2 NeuronCores.

Sharding: 8 cores = 4 batches x 2 fold-halves. Core c handles batch b=c//2
and query blocks {i, 3-i} (1024 rows each) where i=c%2 — the "fold" split
balances causal attention work exactly across the two cores of a batch.

v4: K/V are no longer recomputed for all 4096 keys per core. Each core
projects Q/K/V only for its OWN 2048 rows (4 column blocks), then the two
cores of a batch exchange their K/V halves with a pairwise AllGather
(replica groups [[0,1],[2,3],[4,5],[6,7]]) through a DRAM bounce buffer:

  stage   kt[:,0:1024] + v[:,0:1024]  -> cc_in  (SBUF->DRAM)
  gather  cc_in -> bounce[2,128,2048]         (rank-ordered)
  readback bounce[partner] -> kt_s/v_s[:,2048+...]  (DRAM->SBUF)

The AllGather output is rank-ordered, but every core needs [own|partner]:
the readback issues BOTH candidate DMAs with complementary cond= predicates
driven by a per-core uint32 input (psel), so the program stays SPMD. Two
exchanges pipeline: exchange-1 (own qA keys) hides behind rb2's projection;
exchange-2 (own qB keys) hides behind attn(0,1).

The on-device program is identical on all cores (SPMD); per-core behavior
comes only from the data: the repacked own-rows input xr, psel, and two
slot-bias vectors that enable/disable the two fold-dependent key blocks
(bias 0 keeps scores, bias -50 drives exp() to ~1e-22, i.e. masks).

Other v4 scheduling vs v2: rb3's projection and out-proj qb0..2 ride as
fillers inside the attention units; row-sum trees are 4 pairs deep (1 PE
matmul per 4 pairs, deferred one pair so the PE never waits on the DVE
tree); PE warm-up runs on a memset tile as the first gpsimd op; the final
out-proj stores alternate sync/gpsimd DMA queues.

Layouts (partition dim first):
  xr      [128, 4, 16, 512]  own-rows x[b].T blocks: [p, rb, dt, col]
  Qt      [HD=128, 2048]     own-query projections, head dim on partitions
  Kt, v_s [128, 4096]        [own qA | own qB | partner qA | partner qB]
  scores  [k=128, 1024]      two k-tiles per PSUM pair tile; exp on ACT
  outT    [128, 4, 16, 512]  bf16 [p, qb, dt, col]; host divides by
                             row-sums, transposes back, adds bo
"""

import numpy as np
import ml_dtypes

B, L, D, HD = 4, 4096, 2048, 128
BLK = 1024            # fold block (4 per batch)
LQ = 2 * BLK          # queries per core
LK = L                # keys per core (after exchange)
ND = D // 128         # 16 d-tiles
NRB = LQ // 512       # 4 own column blocks for projections
NEG = -50.0           # slot-disable bias (exp(x-50) ~ 0)
MASKVAL = -30000.0    # intra-tile causal mask additive value

_cached = {}


def _build_program():
    import concourse.bass as bass
    import concourse.tile as tile
    from concourse import bacc, mybir
    from concourse.masks import make_identity

    f32 = mybir.dt.float32
    bf16 = mybir.dt.bfloat16
    u32 = mybir.dt.uint32
    nc = bacc.Bacc("TRN2", target_bir_lowering=False, debug=False,
                   num_devices=8)

    xr_d = nc.dram_tensor("xr", (128, NRB, ND, 512), bf16,
                          kind="ExternalInput")
    wq_d = nc.dram_tensor("wq", (128, ND, 128), bf16, kind="ExternalInput")
    wk_d = nc.dram_tensor("wk", (128, ND, 128), bf16, kind="ExternalInput")
    wv_d = nc.dram_tensor("wv", (128, ND, 128), bf16, kind="ExternalInput")
    wo_d = nc.dram_tensor("wo", (HD, D), bf16, kind="ExternalInput")
    bias_d = nc.dram_tensor("biases", (128, 8), f32, kind="ExternalInput")
    psel_d = nc.dram_tensor("psel", (1, 1), u32, kind="ExternalInput")
    out_d = nc.dram_tensor("outT", (128, 4, ND, 512), bf16,
                           kind="ExternalOutput")
    rs_d = nc.dram_tensor("rowsums", (1, LQ), f32, kind="ExternalOutput")

    GROUPS = [[0, 1], [2, 3], [4, 5], [6, 7]]

    # phase -> list of (local_kblk, kind); kind in {"diag", "full", "bA", "bB"}
    SLOTS = {
        0: [(0, "diag"), (2, "bA")],
        1: [(0, "full"), (1, "diag"), (2, "full"), (3, "bB")],
    }

    with tile.TileContext(nc) as tc:
        with (
            tc.tile_pool(name="const", bufs=1) as cpool,
            tc.tile_pool(name="xt", bufs=4) as xtpool,
            tc.tile_pool(name="vt", bufs=3) as vtpool,
            tc.tile_pool(name="expst", bufs=6) as epool,
            tc.tile_pool(name="outsb", bufs=2) as outpool,
            tc.tile_pool(name="dram", bufs=1, space="DRAM") as dpool,
            tc.tile_pool(name="psum", bufs=1, space="PSUM") as psum,
        ):
            # ---- persistent SBUF tensors ----
            wq_s = cpool.tile([128, ND, 128], bf16, tag="wq")
            wk_s = cpool.tile([128, ND, 128], bf16, tag="wk")
            wv_s = cpool.tile([128, ND, 128], bf16, tag="wv")
            wo_s = cpool.tile([128, D], bf16, tag="wo")
            bias_s = cpool.tile([128, 8], f32, tag="biases")
            kt_s = cpool.tile([128, LK], bf16, tag="kt")
            qt_s = cpool.tile([128, LQ], bf16, tag="qt")
            v_s = cpool.tile([128, LK], bf16, tag="v")
            ones_s = cpool.tile([128, 1], bf16, tag="ones")
            rs_s = cpool.tile([1, LQ], f32, tag="rs")
            masks_s = cpool.tile([128, 4 * 512], bf16, tag="masks")
            ot_s = cpool.tile([128, LQ], bf16, tag="ot")
            identb_s = cpool.tile([128, 128], bf16, tag="identb")
            warm_s = cpool.tile([128, 128], bf16, tag="warm")

            # DRAM bounce buffers for the two K/V pair-exchanges
            cc_in1 = dpool.tile([128, 2048], bf16, tag="cc_in1")
            cc_out1 = dpool.tile([2, 128, 2048], bf16, tag="cc_out1")
            cc_in2 = dpool.tile([128, 2048], bf16, tag="cc_in2")
            cc_out2 = dpool.tile([2, 128, 2048], bf16, tag="cc_out2")

            # wk + first xt chunks first so PE can start K(rb0) ASAP
            xts = {}
            nc.sync.dma_start(wk_s[:], wk_d.ap())
            xts[0] = xtpool.tile([128, ND, 512], bf16, tag="xt", name="xt")
            for ch in range(2):
                # alternate descgen queues so the chunk DMAs don't
                # serialize behind the sync sequencer (~600ns each)
                eng = nc.sync if ch % 2 == 0 else nc.scalar
                eng.dma_start(
                    xts[0][:, ch * 4:(ch + 1) * 4, :],
                    xr_d.ap()[:, 0, ch * 4:(ch + 1) * 4, :],
                )
            nc.sync.dma_start(wv_s[:], wv_d.ap())
            for ch in range(2, 4):
                eng = nc.sync if ch % 2 == 0 else nc.scalar
                eng.dma_start(
                    xts[0][:, ch * 4:(ch + 1) * 4, :],
                    xr_d.ap()[:, 0, ch * 4:(ch + 1) * 4, :],
                )
            nc.scalar.dma_start(bias_s[:], bias_d.ap())
            xts[1] = xtpool.tile([128, ND, 512], bf16, tag="xt", name="xt")
            nc.sync.dma_start(xts[1][:, 0:8, :], xr_d.ap()[:, 1, 0:8, :])
            nc.scalar.dma_start(xts[1][:, 8:16, :], xr_d.ap()[:, 1, 8:16, :])
            nc.sync.dma_start(wq_s[:], wq_d.ap())

            # warm tile memset is the FIRST gpsimd op so the PE_HAM clock
            # gate warm-up (one busy ~4096-cycle window before 2.4GHz
            # releases) can start ~2.5us earlier than behind make_identity
            nc.gpsimd.memset(warm_s[:], 1.0)
            for w in range(24):
                wp = psum.tile([128, 128], f32, tag="acc512", bufs=2,
                               name="warmmm")
                nc.tensor.matmul(wp[:], warm_s[:], warm_s[:],
                                 start=True, stop=True)
            make_identity(nc, identb_s[:])
            nc.gpsimd.memset(ones_s[:], 1.0)
            # preload the ACT exp table during the DMA-bound head (the first
            # real exp would otherwise eat a ~1.3us ACT_TABLE_LOAD stall
            # mid-attention)
            warm = epool.tile([128, 1], bf16, tag="warm", name="warm")
            nc.scalar.activation(
                warm[:], ones_s[:], mybir.ActivationFunctionType.Exp,
                bias=0.0,
            )
            # 4 causal mask tiles for relative offsets delta = 0,128,256,384:
            # keep 0 where q_free >= k_part + delta, else MASKVAL
            nc.gpsimd.memset(masks_s[:], 0.0)
            for m in range(4):
                nc.gpsimd.affine_select(
                    out=masks_s[:, m * 512:(m + 1) * 512],
                    in_=masks_s[:, m * 512:(m + 1) * 512],
                    compare_op=mybir.AluOpType.is_ge,
                    fill=MASKVAL,
                    base=-(m * 128),
                    channel_multiplier=-1,
                    pattern=[[1, 512]],
                )

            # tiny pair-wise AllReduce barrier: absorbs the inter-core
            # launch skew (~10us) during the DMA-bound head so the real
            # K/V exchanges see aligned partners; the readback into barsrc
            # keeps it observable (and blocks only the idle gpsimd queue)
            barsrc = cpool.tile([1, 16], f32, tag="barsrc")
            bar_in = dpool.tile([1, 16], f32, tag="bar_in")
            bar_out = dpool.tile([1, 16], f32, tag="bar_out")
            nc.gpsimd.memset(barsrc[:], 0.0)
            nc.gpsimd.dma_start(bar_in[:], barsrc[:])
            nc.gpsimd.collective_compute(
                "AllReduce",
                mybir.AluOpType.add,
                replica_groups=GROUPS,
                ins=[bar_in[:].opt()],
                outs=[bar_out[:].opt()],
            )
            nc.gpsimd.dma_start(barsrc[:], bar_out[:])

            # psel -> register -> complementary DMA predicates for the
            # rank-ordered AllGather readback (even cores read rank 1,
            # odd cores rank 0; psel holds the partner's rank index)
            preg = nc.gpsimd.alloc_register("psel_reg")
            nc.gpsimd.reg_load(preg, psel_d[0:1, 0:1])
            psel = nc.gpsimd.snap(preg, min_val=0, max_val=1)

            bq_ap = bias_s[:, 0:1]
            bk_ap = bias_s[:, 1:2]
            bv_ap = bias_s[:, 2:3]
            slot_bias = {"bA": bias_s[:, 3:4], "bB": bias_s[:, 4:5]}

            def exchange(half, cc_in, cc_out):
                """Pairwise K/V exchange for own key block `half` (0: qA,
                1: qB). Staging + collective + readback all on the gpsimd
                queue; tile tracks the DRAM deps, and the NRT gpsimd queue
                blocks on collective completion before the readback fires.
                The readback picks the partner's rank-half with a single
                register-indexed DMA (NOT a pair of cond-gated DMAs to the
                same range: tile's last-writer elision would let consumers
                wait only on the second DMA, which on odd cores is the
                skipped one — attention then races the real transfer)."""
                ks = slice(half * 1024, (half + 1) * 1024)
                dst = slice(2048 + half * 1024, 2048 + (half + 1) * 1024)
                nc.gpsimd.dma_start(cc_in[:, 0:1024], kt_s[:, ks])
                nc.gpsimd.dma_start(cc_in[:, 1024:2048], v_s[:, ks])
                nc.gpsimd.collective_compute(
                    "AllGather",
                    mybir.AluOpType.bypass,
                    replica_groups=GROUPS,
                    ins=[cc_in[:].opt()],
                    outs=[cc_out[:].opt()],
                )
                nc.gpsimd.dma_start(kt_s[:, dst], cc_out[psel, :, 0:1024])
                nc.gpsimd.dma_start(v_s[:, dst], cc_out[psel, :, 1024:2048])

            def prefetch(rb):
                # two half-tile DMAs: the consumer's first 8 d-tile matmuls
                # unblock before the full 2MB tile would land
                xts[rb] = xtpool.tile([128, ND, 512], bf16, tag="xt",
                                      name="xt")
                nc.sync.dma_start(xts[rb][:, 0:8, :],
                                  xr_d.ap()[:, rb, 0:8, :])
                nc.scalar.dma_start(xts[rb][:, 8:16, :],
                                    xr_d.ap()[:, rb, 8:16, :])

            def emit_rb_kv(rb, prefetch_rb=None):
                """K and V projections for one 512-wide own column block.
                Q is deferred (not needed until attention) so the K/V halves
                feeding the pair-exchange complete ~20us earlier."""
                xt = xts[rb]
                if prefetch_rb is not None:
                    prefetch(prefetch_rb)
                cs = slice(rb * 512, (rb + 1) * 512)

                pk = psum.tile([128, 512], f32, tag="acc512", bufs=2,
                               name="pk")
                for dt in range(ND):
                    nc.tensor.matmul(
                        pk[:], wk_s[:, dt, :], xt[:, dt, :],
                        start=(dt == 0), stop=(dt == ND - 1),
                    )
                nc.vector.tensor_scalar_add(kt_s[:, cs], pk[:], bk_ap)

                pv = psum.tile([128, 512], f32, tag="acc512", bufs=2,
                               name="pv")
                for dt in range(ND):
                    nc.tensor.matmul(
                        pv[:], wv_s[:, dt, :], xt[:, dt, :],
                        start=(dt == 0), stop=(dt == ND - 1),
                    )
                # drain pv in halves so the first transposes start after
                # ~half the drain latency; all 4 transposes land in ONE bf16
                # PSUM tile, drained by a single DVE copy
                vt_tmp = vtpool.tile([128, 512], bf16, tag="vt_tmp")
                nc.vector.tensor_scalar_add(
                    vt_tmp[:, 0:256], pv[:, 0:256], bv_ap
                )
                nc.vector.tensor_scalar_add(
                    vt_tmp[:, 256:512], pv[:, 256:512], bv_ap
                )
                vp4 = psum.tile([128, 512], bf16, tag="acc512", bufs=2,
                                name="vp4")
                for s in range(4):
                    nc.tensor.transpose(
                        vp4[:, s * 128:(s + 1) * 128],
                        vt_tmp[:, s * 128:(s + 1) * 128],
                        identb_s[:],
                    )
                nc.vector.tensor_copy(v_s[:, cs], vp4[:])

            def emit_q_gen(rb):
                """Deferred Q projection for one 512-query block; rides as
                an attention filler (or runs serial for rb 0)."""
                xt = xts[rb]
                cs = slice(rb * 512, (rb + 1) * 512)
                pq = psum.tile([128, 512], f32, tag="acc512", bufs=2,
                               name="pq")
                for dt in range(ND):
                    nc.tensor.matmul(
                        pq[:], wq_s[:, dt, :], xt[:, dt, :],
                        start=(dt == 0), stop=(dt == ND - 1),
                    )
                    if dt % 4 == 3:
                        yield
                nc.vector.tensor_scalar_add(qt_s[:, cs], pq[:], bq_ap)

            def build_pairs(phase, u):
                """Pairs of k-tiles sharing one exp: (kt_a, kt_b, midx_a,
                bkey). Masked halves use masks_s tile midx_a + h. Pairs are
                emitted in SLOT order, which puts the exchange-dependent
                slots (kblk2/3) last — maximal slack for the collectives.
                The first pair of every unit covers query column 0 (diag
                pairs at u0 have midx 0, so h=0 gets off=0), which the
                start=True AV matmul needs."""
                pairs = []
                for kblk, kind in SLOTS[phase]:
                    tiles = []
                    for t in range(8):
                        if kind == "diag":
                            drel = t * 128 - u * 512
                            if drel >= 512:
                                continue
                            midx = drel // 128 if drel >= 0 else None
                            tiles.append((kblk * 8 + t, midx))
                        else:
                            tiles.append((kblk * 8 + t, None))
                    bkey = kind if kind in slot_bias else None
                    # tiles with masks come in runs of consecutive midx
                    i = 0
                    while i < len(tiles):
                        (ta, ma), (tb, mb) = tiles[i], tiles[i + 1]
                        assert (ma is None) == (mb is None)
                        pairs.append((ta, tb, ma, bkey))
                        i += 2
                return pairs

            def emit_attn_u(phase, u, filler=None, nfill=1, drain=True):
                q0 = phase * BLK + u * 512
                pairs = build_pairs(phase, u)
                n = len(pairs)
                ot_acc = psum.tile([128, 512], f32, tag="otacc", bufs=1,
                                   name="ot_acc")
                rs_acc = psum.tile([1, 512], f32, tag="rs", bufs=1,
                                   name="rs_acc")
                ests = [None] * n

                def emit_pair(pi):
                    ta, tb, ma, bkey = pairs[pi]
                    stp = psum.tile([128, 1024], f32, tag="stp", bufs=2,
                                    name="stp")
                    # causal masks are pre-loaded into PSUM via an identity
                    # matmul and the score matmul accumulates on top — this
                    # keeps the mask off the DVE. The score matmul then only
                    # covers the live query range (the mask-init already
                    # filled the dead zone with MASKVAL, so exp zeroes it).
                    for h, kt in ((0, ta), (1, tb)):
                        if ma is not None:
                            m = ma + h
                            nc.tensor.matmul(
                                stp[:, h * 512:(h + 1) * 512], identb_s[:],
                                masks_s[:, m * 512:(m + 1) * 512],
                                start=True, stop=False,
                                skip_group_check=True,
                            )
                            off = m * 128
                            nc.tensor.matmul(
                                stp[:, h * 512 + off:(h + 1) * 512],
                                kt_s[:, kt * 128:(kt + 1) * 128],
                                qt_s[:, q0 + off:q0 + 512],
                                start=False, stop=True,
                                skip_group_check=True,
                            )
                        else:
                            nc.tensor.matmul(
                                stp[:, h * 512:(h + 1) * 512],
                                kt_s[:, kt * 128:(kt + 1) * 128],
                                qt_s[:, q0:q0 + 512],
                                start=True, stop=True,
                            )
                    est = epool.tile([128, 1024], bf16, tag="est")
                    nc.scalar.activation(
                        est[:], stp[:],
                        mybir.ActivationFunctionType.Exp,
                        bias=slot_bias[bkey] if bkey else 0.0,
                    )
                    ests[pi] = est

                # row-sum grouping: 4-pair groups (1 PE matmul each via a
                # 3-add DVE tree), a possible 2-pair group, and the final
                # 2 pairs always direct (4 rs matmuls, no DVE dependency at
                # the unit boundary so the out-projection starts immediately)
                groups = []
                rem = n - 2
                while rem >= 4:
                    groups.append(4)
                    rem -= 4
                if rem:
                    groups.append(rem)
                groups.append(2)
                gend = []
                acc = 0
                for g in groups:
                    acc += g
                    gend.append(acc)

                emit_pair(0)
                if n > 1:
                    emit_pair(1)
                gi = 0
                pending = None  # (fold_tile, start_flag) deferred rs matmul
                for pi in range(n):
                    ta, tb, ma, bkey = pairs[pi]
                    if pi + 2 < n:
                        emit_pair(pi + 2)
                    if filler is not None:
                        for _ in range(nfill):
                            next(filler, None)
                    est = ests[pi]
                    # masked (diagonal) pairs only contribute on their live
                    # query range; they are ordered last so never carry
                    # start=True (the dead columns were already written by
                    # full pairs)
                    for h, kt in ((0, ta), (1, tb)):
                        off = 0 if ma is None else (ma + h) * 128
                        nc.tensor.matmul(
                            ot_acc[:, off:512],
                            v_s[:, kt * 128:(kt + 1) * 128],
                            est[:, h * 512 + off:(h + 1) * 512],
                            start=(pi == 0 and h == 0),
                            stop=(pi == n - 1 and h == 1),
                            skip_group_check=True,
                        )
                    # the deferred rs matmul from the previous group goes
                    # here (one pair late) so the PE never stalls on the
                    # DVE tree latency
                    if pending is not None:
                        fold, sflag = pending
                        nc.tensor.matmul(
                            rs_acc[:], ones_s[:], fold[:],
                            start=sflag, stop=False,
                        )
                        pending = None
                    if pi + 1 == gend[gi]:
                        gsz = groups[gi]
                        if pi == n - 1:
                            # last group: 4 direct row-sum matmuls
                            for j, (e, h) in enumerate(
                                (e, h)
                                for e in (ests[pi - 1], est)
                                for h in (0, 1)
                            ):
                                nc.tensor.matmul(
                                    rs_acc[:], ones_s[:],
                                    e[:, h * 512:(h + 1) * 512],
                                    start=False, stop=(j == 3),
                                )
                        elif gsz == 2:
                            esum = epool.tile([128, 1024], bf16, tag="esum",
                                              name="esum")
                            nc.vector.tensor_add(
                                esum[:], ests[pi - 1][:], est[:]
                            )
                            fold = epool.tile([128, 512], bf16, tag="fold",
                                              name="fold")
                            nc.vector.tensor_add(
                                fold[:], esum[:, 0:512], esum[:, 512:1024]
                            )
                            pending = (fold, gi == 0)
                        else:  # gsz == 4
                            esA = epool.tile([128, 1024], bf16, tag="esum",
                                             name="esum")
                            nc.vector.tensor_add(
                                esA[:], ests[pi - 3][:], ests[pi - 2][:]
                            )
                            esB = epool.tile([128, 1024], bf16, tag="esumb",
                                             name="esumb")
                            nc.vector.tensor_add(
                                esB[:], ests[pi - 1][:], est[:]
                            )
                            esAB = epool.tile([128, 1024], bf16,
                                              tag="esumab", name="esumab")
                            nc.vector.tensor_add(esAB[:], esA[:], esB[:])
                            fold = epool.tile([128, 512], bf16, tag="fold",
                                              name="fold")
                            nc.vector.tensor_add(
                                fold[:], esAB[:, 0:512], esAB[:, 512:1024]
                            )
                            pending = (fold, gi == 0)
                        gi += 1

                qb = phase * 2 + u
                if phase == 1 and u == 1:
                    # last unit: ACT still has queued exps, a split copy
                    # would finish LATER than a single DVE copy
                    nc.vector.tensor_copy(
                        ot_s[:, qb * 512:(qb + 1) * 512], ot_acc[:]
                    )
                else:
                    # split the u-end ot drain across DVE and ACT so the
                    # next consumer (out-proj matmul) unblocks ~2x sooner
                    nc.vector.tensor_copy(
                        ot_s[:, qb * 512:qb * 512 + 256], ot_acc[:, 0:256]
                    )
                    nc.scalar.activation(
                        ot_s[:, qb * 512 + 256:(qb + 1) * 512],
                        ot_acc[:, 256:512],
                        mybir.ActivationFunctionType.Copy,
                    )
                nc.vector.tensor_copy(
                    rs_s[:, qb * 512:(qb + 1) * 512], rs_acc[:]
                )
                if drain and filler is not None:
                    for _ in filler:  # drain unconsumed filler chunks
                        pass

            def outproj_gen(qb, on_act=False, nstores=2, use_stp=False,
                            alt_queues=False):
                """Out-projection for one 512-query block into a bf16 slab.
                Yields per dt chunk. on_act alternates copies onto ACT (only
                for regions where ACT is not running exp). use_stp borrows
                the stp PSUM banks for a deeper drain pipeline — only safe
                after all attention units are done. alt_queues alternates
                store descgen between the sync and gpsimd queues so the
                tail stores overlap."""
                slab = outpool.tile([128, ND, 512], bf16, tag="oslab",
                                    name="oslab")
                per = ND // nstores
                pop = None
                sidx = 0
                for dt in range(ND):
                    if use_stp and dt % 4 < 2:
                        # alternate between stp pair-tiles and acc512 so
                        # the drain pipeline is 6 deep after attention
                        if dt % 4 == 0:
                            pop = psum.tile([128, 1024], f32, tag="stp",
                                            bufs=2, name="po")
                        po = pop[:, (dt % 4) * 512:(dt % 4) * 512 + 512]
                    else:
                        po = psum.tile([128, 512], f32, tag="acc512",
                                       bufs=2, name="po")[:]
                    nc.tensor.matmul(
                        po,
                        wo_s[:, dt * 128:(dt + 1) * 128],
                        ot_s[:, qb * 512:(qb + 1) * 512],
                        start=True, stop=True,
                    )
                    if on_act and dt % 2 == 1:
                        nc.scalar.activation(
                            slab[:, dt, :], po,
                            mybir.ActivationFunctionType.Copy,
                        )
                    else:
                        nc.vector.tensor_copy(slab[:, dt, :], po)
                    if dt % per == per - 1:
                        s = dt + 1 - per
                        eng = nc.gpsimd if (alt_queues and sidx % 2) \
                            else nc.sync
                        eng.dma_start(
                            out_d.ap()[:, qb, s:dt + 1], slab[:, s:dt + 1]
                        )
                        sidx += 1
                    yield

            def emit_outproj(qb, on_act=False, nstores=2, use_stp=False,
                             alt_queues=False):
                for _ in outproj_gen(qb, on_act, nstores, use_stp,
                                     alt_queues):
                    pass

            def chain(*gens):
                for g in gens:
                    for x in g:
                        yield x

            # ---- schedule ----
            # K/V-only projections run back-to-back (Q deferred), so
            # exchange-1 (own qA K/V) is staged at ~rb1 end and its
            # collective hides behind rb2+rb3+Q0; exchange-2 hides behind
            # attn(0,*) (its consumer, the kblk3 slot, comes last in
            # phase 1). high_priority makes the tile scheduler place the
            # exchange instructions as early as their deps allow. Deferred
            # Q1..3 and out-projections qb0..2 ride as attention fillers;
            # only qb3's out-projection remains for the tail.
            emit_rb_kv(0, prefetch_rb=2)
            nc.sync.dma_start(wo_s[:], wo_d.ap())
            emit_rb_kv(1, prefetch_rb=3)
            with tc.high_priority():
                exchange(0, cc_in1, cc_out1)
            emit_rb_kv(2)
            emit_rb_kv(3)
            with tc.high_priority():
                exchange(1, cc_in2, cc_out2)
            for _ in emit_q_gen(0):
                pass
            f = chain(
                emit_q_gen(1),
                emit_q_gen(2),
                emit_q_gen(3),
                outproj_gen(0, on_act=True),
                outproj_gen(1),
                outproj_gen(2),
            )
            emit_attn_u(0, 0, filler=f, drain=False)
            emit_attn_u(0, 1, filler=f, drain=False)
            emit_attn_u(1, 0, filler=f, drain=False)
            emit_attn_u(1, 1, filler=f, drain=False)
            # leftover filler chunks (tail of outproj 2) drain here
            for _ in f:
                pass
            nc.sync.dma_start(rs_d.ap(), rs_s[:])
            emit_outproj(3, on_act=True, nstores=8, use_stp=True,
                         alt_queues=True)

    nc.compile()
    return nc


def _get_program():
    if "nc" not in _cached:
        _cached["nc"] = _build_program()
    return _cached["nc"]


def _perm_blocks(i):
    # local order [qA, qB, o1, o2]
    return [0, 3, 1, 2] if i == 0 else [1, 2, 0, 3]


def _repack_w(w):
    # (D, HD) -> [128, ND, 128] with per-partition contiguous lines
    return np.ascontiguousarray(
        w.reshape(ND, 128, HD).transpose(1, 0, 2)
    ).astype(ml_dtypes.bfloat16)


def make_in_maps(x, Wq, bq, Wk, bk, Wv, bv, Wo, bo):
    scale = 1.0 / np.sqrt(np.float32(HD))
    wq_r = _repack_w((Wq * scale).astype(np.float32))
    wk_r = _repack_w(Wk.astype(np.float32))
    wv_r = _repack_w(Wv.astype(np.float32))
    bq_s = (bq * scale).astype(np.float32)
    in_maps = []
    for c in range(8):
        i, b = c % 2, c // 2
        perm = _perm_blocks(i)
        xbT = x[b].T  # (D, L) view
        # own two query blocks only (2048 columns)
        xT = np.concatenate(
            [xbT[:, p * BLK:(p + 1) * BLK] for p in perm[:2]], axis=1
        )
        # (D, LQ) -> [128, NRB, ND, 512]: xr[p, rb, dt, c] = xT[dt*128+p,
        # rb*512+c]
        xr = np.ascontiguousarray(
            xT.reshape(ND, 128, NRB, 512).transpose(1, 2, 0, 3)
        ).astype(ml_dtypes.bfloat16)
        biases = np.zeros((128, 8), np.float32)
        biases[:, 0] = bq_s
        biases[:, 1] = bk.astype(np.float32)
        biases[:, 2] = bv.astype(np.float32)
        biases[:, 3] = NEG if i == 0 else 0.0   # phase A, slot kblk=2
        biases[:, 4] = 0.0 if i == 0 else NEG   # phase B, slot kblk=3
        in_maps.append({
            "xr": xr,
            "wq": wq_r,
            "wk": wk_r,
            "wv": wv_r,
            "wo": Wo.astype(ml_dtypes.bfloat16),
            "biases": biases,
            # partner's rank index inside the AllGather pair group
            "psel": np.array([[1 - (c % 2)]], dtype=np.uint32),
        })
    return in_maps


def assemble_output(results, bo):
    out = np.empty((B, L, D), np.float32)
    for c in range(8):
        i, b = c % 2, c // 2
        perm = _perm_blocks(i)
        arr = np.asarray(results[c]["outT"], dtype=np.float32)
        # [128, 4, ND, 512] -> (D, LQ)
        outT = arr.transpose(2, 0, 1, 3).reshape(D, LQ)
        outT /= np.asarray(results[c]["rowsums"], dtype=np.float32)
        qA, qB = perm[0], perm[1]
        out[b, qA * BLK:(qA + 1) * BLK, :] = outT[:, 0:BLK].T
        out[b, qB * BLK:(qB + 1) * BLK, :] = outT[:, BLK:2 * BLK].T
    out += bo.astype(np.float32)
    return out


def kernel(x, Wq, bq, Wk, bk, Wv, bv, Wo, bo):
    from concourse.bass_utils import run_bass_kernel_spmd

    nc = _get_program()
    in_maps = make_in_maps(
        np.asarray(x), np.asarray(Wq), np.asarray(bq), np.asarray(Wk),
        np.asarray(bk), np.asarray(Wv), np.asarray(bv), np.asarray(Wo),
        np.asarray(bo),
    )
    res = run_bass_kernel_spmd(nc, in_maps, core_ids=list(range(8)))
    return assemble_output(res.results, np.asarray(bo))
